# revision 1
# baseline (speedup 1.0000x reference)
"""Trainium2 Bass kernel for nn_DecoderCell_59742995087471.

Decoder cell: causal self-attention + add&LN, cross-attention over H + add&LN,
single-Linear FFN + add&LN.  B=2, S=T=2048, D=1024, 16 heads x 64.

Sharding (no collectives): 8 cores = 2 batch elements x 4 query-blocks of 512
rows.  Each core redundantly computes the K/V projections for its batch element
(from the full S0 / H, which the host replicates per batch) and runs attention +
FFN + all three LayerNorms for its own 512 query rows.  The causal mask arrives
as data ([2048, 512] multiplicative 0/1, applied post-exp), so the instruction
stream is identical on every core (SPMD).

Layout: activations are kept transposed in SBUF ([d on partitions, rows free]).
All matmul operands are bf16 (fp32 PSUM accumulate); residuals and LN math stay
fp32.  Softmax has no max-subtraction (scores are O(1) for this problem's data
scale: weights ~N(0, 0.02^2), activations ~N(0,1), so |scores/8| < ~4) and the
denominator comes free from a ones-augmented column in the PV matmul.
"""

import numpy as np
import ml_dtypes

import concourse.bass as bass
import concourse.bacc as bacc
import concourse.mybir as mybir
import concourse.tile as tile

F32 = mybir.dt.float32
BF16 = mybir.dt.bfloat16
AF = mybir.ActivationFunctionType
ALU = mybir.AluOpType

B, S, D, H, HD = 2, 2048, 1024, 16, 64
QL = 512          # query rows per core
NC = 8            # cores
DT = D // 128     # 8 d-tiles
KT = S // 128     # 16 key tiles
PAIRS = H // 2    # 8 head pairs
EPS = 1e-5

W_NAMES = ["Wq1", "Wk1", "Wv1", "Wo1", "Wq2", "Wk2", "Wv2", "Wo2", "Wf"]
# par columns (per-partition params, [1024, NPAR] fp32)
PC = {"bq1": 0, "bk1": 1, "bo1": 2, "g1": 3, "b1": 4,
      "bq2": 5, "bk2": 6, "bo2": 7, "g2": 8, "b2": 9,
      "bf": 10, "g3": 11, "b3": 12}
NPAR = 13

BUFS = {
    "xt": 8,     # [128,2048] bf16: x0t -> ht rotation
    "kT": 4,     # [128,2048] bf16: K pairs stream through attn
    "v": 16,     # [128,1040] bf16: V1 -> V2 rotation
    "qT": 8,     # [128,512] bf16 Q^T pairs
    "sb16": 10,  # [128,512] bf16: x0q, s1, s2
    "res": 10,   # [128,512] f32 residual stream generations
    "xpre": 2, "xsq": 2,
    "m": 6,      # mask tiles
    "p": 5,      # [128,1024] bf16 probs
    "o": 8,      # [128,512] bf16 oT pairs
    "w": 10,    # [128,1024] bf16 weights (non-K)
    "wk": 8,    # [128,1024] bf16 K-proj weights (long-lived: filler reads)
    "sm": 3,     # [1,512] f32 smalls
    "smb": 2,    # [128,512] f32 broadcasts
    "rb": 2,     # [64,512] f32
    "t1": 2, "t2": 2,  # [128,512] f32 LN temps
}


def _build_body(nc, tc, d, ctx):
    pools = {}

    def _pool(tag, bufs, space="SBUF"):
        if tag not in pools:
            pools[tag] = ctx.enter_context(
                tc.tile_pool(name=tag, bufs=bufs, space=space))
        return pools[tag]

    # create every pool up front (before any instruction is emitted)
    for tag, bufs in BUFS.items():
        _pool(tag, bufs)
    for dt_ in range(DT):
        _pool(f"par{dt_}", 1)
    for tag in ("ones", "eps"):
        _pool(tag, 1)
    for tag, bufs in (("acc", 2), ("pv", 2), ("sc", 2)):
        _pool("ps_" + tag, bufs, space="PSUM")

    def sbt(shape, dtype, tag):
        return _pool(tag, BUFS[tag]).tile(shape, dtype, tag=tag, name=tag)

    class _PS:
        @staticmethod
        def tile(shape, dtype, tag, bufs, name):
            return _pool("ps_" + tag, bufs, space="PSUM").tile(
                shape, dtype, tag=tag, name=name)
    PS = _PS()

    class _SB:
        @staticmethod
        def tile(shape, dtype, tag, bufs, name):
            return _pool(tag, bufs).tile(shape, dtype, tag=tag, name=name)
    SB = _SB()

    # ---------------- constants / params ----------------
    par_t = []
    for dt_ in range(DT):
        pt = SB.tile([128, NPAR], F32, tag=f"par{dt_}", bufs=1, name=f"par{dt_}")
        nc.sync.dma_start(pt, d["par"][dt_ * 128:(dt_ + 1) * 128, :])
        par_t.append(pt)
    ones_t = SB.tile([128, 1], BF16, tag="ones", bufs=1, name="ones")
    nc.vector.memset(ones_t, 1.0)
    eps_t = SB.tile([1, 1], F32, tag="eps", bufs=1, name="eps")
    nc.vector.memset(eps_t, EPS)

    def pap(dt_, key):
        c = PC[key]
        return par_t[dt_][:, c:c + 1]

    # ---------------- input loads ----------------
    # queue order matters: x0q + Wq1 first so Q1 can start ~9us in; the bulk
    # loads follow and overlap Q1 compute.
    x0q = []
    wq1 = []
    for dt_ in range(DT):
        t = sbt([128, QL], BF16, "sb16")
        nc.sync.dma_start(t, d["x0q"][dt_ * 128:(dt_ + 1) * 128, :])
        x0q.append(t)
        t = sbt([128, D], BF16, "w")
        nc.sync.dma_start(t, d["Wq1"][dt_ * 128:(dt_ + 1) * 128, :])
        wq1.append(t)

    def load_w(name, tag="w"):
        tiles = []
        for dt_ in range(DT):
            t = sbt([128, D], BF16, tag)
            nc.sync.dma_start(t, d[name][dt_ * 128:(dt_ + 1) * 128, :])
            tiles.append(t)
        return tiles

    # ---------------- building blocks ----------------
    def proj_pair_unit(w_t, x_t, out_tile, pair, c0, bias_ap, copy_dve=False):
        """out_tile[:, c0:c0+512] (bf16) = W[:, pair].T @ x[:, c0:c0+512] + bias"""
        acc = PS.tile([128, 512], F32, tag="acc", bufs=2, name="acc")
        for dt_ in range(DT):
            nc.tensor.matmul(acc, w_t[dt_][:, pair * 128:(pair + 1) * 128],
                             x_t[dt_][:, c0:c0 + 512],
                             start=(dt_ == 0), stop=(dt_ == DT - 1))
        if copy_dve:
            # inside attention ACT is the pacing engine - keep copies off it
            nc.vector.tensor_scalar(out_tile[:, c0:c0 + 512], acc, bias_ap,
                                    None, op0=ALU.add)
        else:
            nc.scalar.activation(out_tile[:, c0:c0 + 512], acc, AF.Identity,
                                 bias=bias_ap)

    def v_unit(w_t, x_t, vtile, kt_, half):
        """vtile heads [half*8:(half+1)*8] cols = x[:, kt].T @ W[:, half*512:...]"""
        acc = PS.tile([128, 512], F32, tag="acc", bufs=2, name="acc")
        for dt_ in range(DT):
            nc.tensor.matmul(acc, x_t[dt_][:, kt_ * 128:(kt_ + 1) * 128],
                             w_t[dt_][:, half * 512:(half + 1) * 512],
                             start=(dt_ == 0), stop=(dt_ == DT - 1))
        vv = vtile.rearrange("p (h c) -> p h c", h=H)[:, half * 8:(half + 1) * 8, 0:HD]
        av = acc.rearrange("p (h c) -> p h c", h=8)
        nc.vector.tensor_copy(vv, av)

    def emit_k_pair(wk, x_t, bk_key, pair, copy_dve=False):
        kt_t = sbt([128, S], BF16, "kT")
        for c in range(S // 512):
            proj_pair_unit(wk, x_t, kt_t, pair, c * 512, pap(pair, bk_key),
                           copy_dve=copy_dve)
        return kt_t

    def emit_q_all(wq, xq_t, bq_key):
        qT = []
        for pair in range(PAIRS):
            qt = sbt([128, QL], BF16, "qT")
            proj_pair_unit(wq, xq_t, qt, pair, 0, pap(pair, bq_key))
            qT.append(qt)
        return qT

    def emit_v_all(wv, x_t):
        v_ = []
        for kt_ in range(KT):
            vt = sbt([128, H * (HD + 1)], BF16, "v")
            nc.vector.memset(
                vt.rearrange("p (h c) -> p h c", h=H)[:, :, HD:HD + 1], 1.0)
            for half in range(2):
                v_unit(wv, x_t, vt, kt_, half)
            v_.append(vt)
        return v_

    def make_k(wk, x_t, bk_key, n_upfront):
        kT_ = [emit_k_pair(wk, x_t, bk_key, p) for p in range(n_upfront)]

        def k_filler(pair_done):
            nxt = len(kT_)
            if nxt < PAIRS and nxt <= pair_done + 2:
                kT_.append(emit_k_pair(wk, x_t, bk_key, nxt, copy_dve=True))
        return kT_, k_filler

    def emit_attn(kT_t, v_t, qT_t, mask_d, k_filler=None):
        """Returns 8 assembled oT pair tiles ([128, 512] bf16)."""
        oT_pairs = []
        for pair in range(PAIRS):
            pvs = [PS.tile([HD + 1, QL], F32, tag="pv", bufs=2, name="pv")
                   for _ in range(2)]
            prev = None  # (pT, kt) pending PV
            for kt_ in range(KT):
                if mask_d is not None:
                    mt = sbt([128, QL], BF16, "m")
                    nc.sync.dma_start(mt, mask_d[kt_ * 128:(kt_ + 1) * 128, :])
                psc = PS.tile([128, 2 * QL], F32, tag="sc", bufs=2, name="sc")
                for half in range(2):
                    nc.tensor.matmul(
                        psc[:, half * QL:(half + 1) * QL],
                        kT_t[pair][half * HD:(half + 1) * HD,
                                   kt_ * 128:(kt_ + 1) * 128],
                        qT_t[pair][half * HD:(half + 1) * HD, :],
                        start=True, stop=True)
                pT = sbt([128, 2 * QL], BF16, "p")
                nc.scalar.activation(pT, psc, AF.Exp, scale=0.125)
                if mask_d is not None:
                    for half in range(2):
                        nc.vector.tensor_mul(pT[:, half * QL:(half + 1) * QL],
                                             pT[:, half * QL:(half + 1) * QL], mt)
                if prev is not None:
                    ppT, pkt = prev
                    for half in range(2):
                        h = pair * 2 + half
                        nc.tensor.matmul(
                            pvs[half], v_t[pkt][:, h * (HD + 1):h * (HD + 1) + HD + 1],
                            ppT[:, half * QL:(half + 1) * QL],
                            start=(pkt == 0), stop=(pkt == KT - 1),
                            skip_group_check=True)
                prev = (pT, kt_)
            ppT, pkt = prev
            for half in range(2):
                h = pair * 2 + half
                nc.tensor.matmul(
                    pvs[half], v_t[pkt][:, h * (HD + 1):h * (HD + 1) + HD + 1],
                    ppT[:, half * QL:(half + 1) * QL],
                    start=(pkt == 0), stop=(pkt == KT - 1), skip_group_check=True)
            if k_filler is not None:
                k_filler(pair)
            oT = sbt([128, QL], BF16, "o")
            for half in range(2):
                recip = sbt([1, QL], F32, "sm")
                nc.vector.reciprocal(recip, pvs[half][HD:HD + 1, :])
                rb = sbt([HD, QL], F32, "rb")
                nc.gpsimd.partition_broadcast(rb, recip)
                nc.vector.tensor_mul(oT[half * HD:(half + 1) * HD, :],
                                     pvs[half][0:HD, :], rb)
            oT_pairs.append(oT)
        return oT_pairs

    def emit_out_proj(w_t, in_pairs, bias_key, resid_t):
        """pre[dt] (f32) = W.T @ in_pairs + bias + resid"""
        pre = []
        for m in range(DT):
            acc = PS.tile([128, 512], F32, tag="acc", bufs=2, name="acc")
            for pr in range(PAIRS):
                nc.tensor.matmul(acc, w_t[pr][:, m * 128:(m + 1) * 128],
                                 in_pairs[pr],
                                 start=(pr == 0), stop=(pr == PAIRS - 1))
            t = sbt([128, QL], F32, "res")
            nc.vector.scalar_tensor_tensor(t, acc, pap(m, bias_key), resid_t[m],
                                           op0=ALU.add, op1=ALU.add)
            pre.append(t)
        return pre

    def emit_ln(pre_t, g_key, b_key, want_bf16):
        xb, xq_ = [], []
        for dt_ in range(DT):
            t = sbt([128, QL], BF16, "xpre")
            nc.vector.tensor_copy(t, pre_t[dt_])
            xb.append(t)
            t2_ = sbt([128, QL], BF16, "xsq")
            nc.scalar.square(t2_, pre_t[dt_])
            xq_.append(t2_)
        sx = PS.tile([1, QL], F32, tag="acc", bufs=2, name="acc")
        for dt_ in range(DT):
            nc.tensor.matmul(sx, ones_t, xb[dt_], start=(dt_ == 0),
                             stop=(dt_ == DT - 1), skip_group_check=True)
        sxx = PS.tile([1, QL], F32, tag="acc", bufs=2, name="acc")
        for dt_ in range(DT):
            nc.tensor.matmul(sxx, ones_t, xq_[dt_], start=(dt_ == 0),
                             stop=(dt_ == DT - 1), skip_group_check=True)
        mean = sbt([1, QL], F32, "sm")
        nc.vector.tensor_scalar(mean, sx, 1.0 / D, None, op0=ALU.mult)
        meanb = sbt([128, QL], F32, "smb")
        nc.gpsimd.partition_broadcast(meanb, mean)
        msq = sbt([1, QL], F32, "sm")
        nc.vector.tensor_mul(msq, mean, mean)
        var = sbt([1, QL], F32, "sm")
        nc.vector.scalar_tensor_tensor(var, sxx, 1.0 / D, msq,
                                       op0=ALU.mult, op1=ALU.subtract)
        sd = sbt([1, QL], F32, "sm")
        nc.scalar.activation(sd, var, AF.Sqrt, bias=eps_t)
        rstd = sbt([1, QL], F32, "sm")
        nc.vector.reciprocal(rstd, sd)
        rstdb = sbt([128, QL], F32, "smb")
        nc.gpsimd.partition_broadcast(rstdb, rstd)
        out32, out16 = [], []
        for dt_ in range(DT):
            t1 = sbt([128, QL], F32, "t1")
            nc.vector.tensor_sub(t1, pre_t[dt_], meanb)
            t2_ = sbt([128, QL], F32, "t2")
            nc.vector.tensor_mul(t2_, t1, rstdb)
            o32 = sbt([128, QL], F32, "res")
            nc.vector.tensor_scalar(o32, t2_, pap(dt_, g_key), pap(dt_, b_key),
                                    op0=ALU.mult, op1=ALU.add)
            out32.append(o32)
            if want_bf16:
                o16 = sbt([128, QL], BF16, "sb16")
                nc.vector.tensor_scalar(o16, t2_, pap(dt_, g_key),
                                        pap(dt_, b_key), op0=ALU.mult, op1=ALU.add)
                out16.append(o16)
        return out32, out16

    # ---------------- the decoder cell ----------------
    import os
    stop_after = os.environ.get("KSTOP", "")

    def _early_out(tiles):
        for dt_ in range(DT):
            nc.sync.dma_start(d["out"][dt_ * 128:(dt_ + 1) * 128, :], tiles[dt_])
        return True

    x0t = []
    for dt_ in range(DT):
        t = sbt([128, S], BF16, "xt")
        nc.sync.dma_start(t, d["x0t"][dt_ * 128:(dt_ + 1) * 128, :])
        x0t.append(t)
    wv1 = load_w("Wv1"); wk1 = load_w("Wk1", tag="wk")
    q1 = emit_q_all(wq1, x0q, "bq1")
    v1 = emit_v_all(wv1, x0t)
    k1, kf1 = make_k(wk1, x0t, "bk1", 2)
    if stop_after == "qkv1":
        _early_out(x0r); return

    # ht loads reuse x0t slots (dead after QKV1)
    ht = []
    for dt_ in range(DT):
        t = sbt([128, S], BF16, "xt")
        nc.sync.dma_start(t, d["ht"][dt_ * 128:(dt_ + 1) * 128, :])
        ht.append(t)

    x0r = []
    for dt_ in range(DT):
        t = sbt([128, QL], F32, "res")
        nc.sync.dma_start(t, d["x0r"][dt_ * 128:(dt_ + 1) * 128, :])
        x0r.append(t)
    o1 = emit_attn(k1, v1, q1, d["msk"], k_filler=kf1)
    if stop_after == "attn1":
        _early_out(x0r); return

    wv2 = load_w("Wv2")
    v2 = emit_v_all(wv2, ht)

    wo1 = load_w("Wo1")
    pre1 = emit_out_proj(wo1, o1, "bo1", x0r)
    if stop_after == "wo1":
        _early_out(pre1); return
    s1_32, s1_16 = emit_ln(pre1, "g1", "b1", want_bf16=True)
    if stop_after == "ln1":
        _early_out(s1_32); return

    wk2 = load_w("Wk2", tag="wk"); wq2 = load_w("Wq2")
    k2, kf2 = make_k(wk2, ht, "bk2", 2)
    q2 = emit_q_all(wq2, s1_16, "bq2")
    if stop_after == "qkv2":
        _early_out(s1_32); return

    o2 = emit_attn(k2, v2, q2, None, k_filler=kf2)
    if stop_after == "attn2":
        _early_out(s1_32); return

    wo2 = load_w("Wo2")
    pre2 = emit_out_proj(wo2, o2, "bo2", s1_32)
    s2_32, s2_16 = emit_ln(pre2, "g2", "b2", want_bf16=True)

    wf = load_w("Wf")
    pre3 = emit_out_proj(wf, s2_16, "bf", s2_32)
    s3_32, _ = emit_ln(pre3, "g3", "b3", want_bf16=False)

    for dt_ in range(DT):
        nc.sync.dma_start(d["out"][dt_ * 128:(dt_ + 1) * 128, :], s3_32[dt_])


_CACHE = {}


def build_program():
    if "nc" in _CACHE:
        return _CACHE["nc"]
    nc = bacc.Bacc("TRN2", target_bir_lowering=False, debug=False, num_devices=NC)
    d = {}
    d["x0t"] = nc.dram_tensor("x0t", [D, S], BF16, kind="ExternalInput")
    d["ht"] = nc.dram_tensor("ht", [D, S], BF16, kind="ExternalInput")
    d["x0q"] = nc.dram_tensor("x0q", [D, QL], BF16, kind="ExternalInput")
    d["x0r"] = nc.dram_tensor("x0r", [D, QL], F32, kind="ExternalInput")
    d["msk"] = nc.dram_tensor("msk", [S, QL], BF16, kind="ExternalInput")
    for w in W_NAMES:
        d[w] = nc.dram_tensor(w, [D, D], BF16, kind="ExternalInput")
    d["par"] = nc.dram_tensor("par", [D, NPAR], F32, kind="ExternalInput")
    d["out"] = nc.dram_tensor("out", [D, QL], F32, kind="ExternalOutput")

    from contextlib import ExitStack
    with tile.TileContext(nc) as tc:
        with ExitStack() as ctx:
            _build_body(nc, tc, {k: (v[:] if hasattr(v, "ap") else v)
                                 for k, v in d.items()}, ctx)
    nc.compile()
    _CACHE["nc"] = nc
    return nc


def make_in_maps(inputs):
    """Build the 8 per-core input dicts from the full problem inputs."""
    bf = ml_dtypes.bfloat16
    S0 = np.asarray(inputs["S0"], np.float32)
    Hh = np.asarray(inputs["H"], np.float32)

    par = np.zeros((D, NPAR), np.float32)
    for key, col in PC.items():
        src = {"bq1": "bq1", "bk1": "bk1", "bo1": "bo1", "g1": "ln1_g",
               "b1": "ln1_b", "bq2": "bq2", "bk2": "bk2", "bo2": "bo2",
               "g2": "ln2_g", "b2": "ln2_b", "bf": "bf", "g3": "ln3_g",
               "b3": "ln3_b"}[key]
        par[:, col] = np.asarray(inputs[src], np.float32)
    # bv folds exactly into bo: a = (o + bv) @ Wo + bo = o @ Wo + (bv @ Wo + bo)
    par[:, PC["bo1"]] += np.asarray(inputs["bv1"], np.float32) @ np.asarray(
        inputs["Wo1"], np.float32)
    par[:, PC["bo2"]] += np.asarray(inputs["bv2"], np.float32) @ np.asarray(
        inputs["Wo2"], np.float32)

    ws = {w: np.ascontiguousarray(np.asarray(inputs[w], np.float32)).astype(bf)
          for w in W_NAMES}

    in_maps = []
    for c in range(NC):
        b, j = c // 4, c % 4
        q0 = j * QL
        x0t = np.ascontiguousarray(S0[b].T)
        ht = np.ascontiguousarray(Hh[b].T)
        mask = (np.arange(S)[:, None] <= (q0 + np.arange(QL))[None, :])
        m = {
            "x0t": x0t.astype(bf),
            "ht": ht.astype(bf),
            "x0q": np.ascontiguousarray(x0t[:, q0:q0 + QL]).astype(bf),
            "x0r": np.ascontiguousarray(x0t[:, q0:q0 + QL]),
            "msk": mask.astype(bf),
            "par": par,
        }
        m.update(ws)
        in_maps.append(m)
    return in_maps


def kernel(**inputs) -> np.ndarray:
    from concourse.bass_utils import run_bass_kernel_spmd
    nc = build_program()
    in_maps = make_in_maps(inputs)
    res = run_bass_kernel_spmd(nc, in_maps, list(range(NC)))
    _CACHE["last_results"] = res
    out = np.zeros((B, S, D), np.float32)
    for c in range(NC):
        b, j = c // 4, c % 4
        out[b, j * QL:(j + 1) * QL, :] = res.results[c]["out"].T
    return out



# revision 10
# speedup vs baseline: 1.1643x; 1.1643x over previous
"""Trainium2 Bass kernel for nn_DecoderCell_59742995087471 (fp8 DoubleRow).

Decoder cell: causal self-attention + add&LN, cross-attention over H + add&LN,
single-Linear FFN + add&LN.  B=2, S=T=2048, D=1024, 16 heads x 64.

Sharding (no collectives): 8 cores = 2 batch elements x 4 query-blocks of 512
rows.  Each core redundantly computes K/V projections for its batch element
and runs attention + FFN + LNs for its own 512 query rows.

Fast path: Q/K/V/Wo projections and PV run as fp8e4m3 DoubleRow matmuls
(2 k-subtiles per instruction, 0.5 cyc/row = 4x bf16 MACs).  Scores stay
bf16.  Scaling keeps everything in fp8 normal range:
  x8 = 4*x, W8 = 32*W  -> q/k = psum/128 (bf16)
  v8 = psum/4 = 32*v   -> pv_psum = sum(p8*v8) with p8 = exp(s)/4
  denom row (ones in v8) = sum(p)/4, recip = 4/sum(p)
  o8 = pv_psum * recip_bcast = 32*o ;  Wo8 = 32*Wo -> psum/1024 = a
Biases: bv folds into bo (host), bo1 folds into the x0 residual (host),
bo2 folds into LN1's beta column (host).  FFN stays bf16.

Causal masking is made SPMD-uniform by a per-core KEY PERMUTATION (host):
each core's K/V key order puts its own 512-row diagonal block first, then
the fully-visible keys, then the fully-masked keys.  The triangular mask
multiply then always applies to kt 0..3, and a per-core bias table feeds
exp's per-partition bias: -ln4 for visible kt, -100 (=> exp 0) for masked.
"""

import numpy as np
import ml_dtypes

import concourse.bass as bass
import concourse.bacc as bacc
import concourse.mybir as mybir
import concourse.tile as tile

F32 = mybir.dt.float32
F32R = mybir.dt.float32r
BF16 = mybir.dt.bfloat16
FP8 = mybir.dt.float8e4
AF = mybir.ActivationFunctionType
ALU = mybir.AluOpType
DR = mybir.MatmulPerfMode.DoubleRow

B, S, D, H, HD = 2, 2048, 1024, 16, 64
QL = 512          # query rows per core
NC = 8            # cores
XT = 4            # x/w tiles of 256 contraction rows (DoubleRow pairs)
DT = 8            # 128-row d-tiles (outputs)
KT = S // 128     # 16 key tiles
KP = KT // 2      # 8 key-tile pairs
PAIRS = H // 2    # 8 head pairs
EPS = 1e-5
MLN4 = -1.3862943611198906  # -ln(4)

W8_NAMES = ["Wq1", "Wk1", "Wv1", "Wo1", "Wq2", "Wk2", "Wv2", "Wo2"]
# par columns (per-partition params, [1024, NPAR] fp32)
PC = {"bq1": 0, "bk1": 1, "g1": 2, "b1o": 3, "g1x4": 4, "b1x4": 5,
      "bq2": 6, "bk2": 7, "g2": 8, "b2": 9, "bf": 10, "g3": 11, "b3": 12}
NPAR = 13

BUFS = {
    "xt": 8,     # [128,4096] fp8 DR tiles: x8 -> h8 rotation (4KB each)
    "kT": 4,     # [128,2048] bf16 K pairs
    "v": 16,     # [128,2080] fp8 ktpair V tiles (2 attns)
    "qT": 8,     # [128,512] bf16
    "res": 10,   # [128,512] f32 residual stream
    "s18": 4,    # [128,1024] fp8 DR s1 tiles
    "sb16": 8,   # [128,512] bf16 (s2_16)
    "xsq": 2,    # [128,512] bf16 squares
    "m": 4,      # [128,1024] fp8 masks
    "p": 4,      # [128,2048] fp8 probs (ktpair)
    "o": 4,      # [128,1024] fp8 o-pair DR tiles
    "w": 18,     # [128,2048] fp8 W tiles (4 per W) + bf16 Wf [128,1024]
    "sm": 4,     # [1,512] f32 smalls
    "smb": 2,    # [128,512] f32 broadcasts
    "rb": 2,     # [64,512] f32
    "t1": 2, "t2": 2,  # [128,512] f32 LN temps
    "bt": 1,     # [128,16] f32 exp-bias table
}


def _build_body(nc, tc, d, ctx):
    pools = {}

    def _pool(tag, bufs, space="SBUF"):
        if tag not in pools:
            pools[tag] = ctx.enter_context(
                tc.tile_pool(name=tag, bufs=bufs, space=space))
        return pools[tag]

    for tag, bufs in BUFS.items():
        _pool(tag, bufs)
    for dt_ in range(DT):
        _pool(f"par{dt_}", 1)
    for tag in ("ones", "onesr", "eps", "mln4"):
        _pool(tag, 1)
    for tag, bufs in (("acc", 2), ("pv", 2), ("sc", 2)):
        _pool("ps_" + tag, bufs, space="PSUM")

    def sbt(shape, dtype, tag):
        return _pool(tag, BUFS[tag]).tile(shape, dtype, tag=tag, name=tag)

    def pst(shape, tag):
        return _pool("ps_" + tag, 2, space="PSUM").tile(
            shape, F32, tag=tag, name=tag)

    def one_t(tag, shape, dtype):
        return _pool(tag, 1).tile(shape, dtype, tag=tag, name=tag)

    # ---------------- constants / params ----------------
    par_t = []
    for dt_ in range(DT):
        pt = one_t(f"par{dt_}", [128, NPAR], F32)
        nc.sync.dma_start(pt, d["par"][dt_ * 128:(dt_ + 1) * 128, :])
        par_t.append(pt)
    ones_t = one_t("ones", [128, 1], BF16)
    nc.vector.memset(ones_t, 1.0)
    onesr_t = one_t("onesr", [128, 1], F32R)
    nc.sync.dma_start(onesr_t, d["onesr"])
    eps_t = one_t("eps", [1, 1], F32)
    nc.vector.memset(eps_t, EPS)
    mln4_t = one_t("mln4", [128, 1], F32)
    nc.vector.memset(mln4_t, MLN4)
    bt_t = one_t("bt", [128, KT], F32)
    nc.sync.dma_start(bt_t, d["btbl"])

    def pap(dt_, key):
        return par_t[dt_][:, PC[key]:PC[key] + 1]

    # ---------------- input loads ----------------
    # x8 query-slice first (cols 0:512 are this core's own rows thanks to
    # the key permutation) so Q1 can start early; bulk keys follow.
    x8 = []
    for t in range(XT):
        xt_ = sbt([128, 2 * S], FP8, "xt")
        for i in range(2):
            nc.sync.dma_start(xt_[:, i * S:i * S + QL],
                              d["x8"][256 * t + 128 * i:256 * t + 128 * i + 128, 0:QL])
        x8.append(xt_)
    w8 = {}

    def load_w8(name):
        tiles = []
        for t in range(XT):
            wt = sbt([128, 2 * D], FP8, "w")
            for i in range(2):
                nc.sync.dma_start(wt[:, i * D:(i + 1) * D],
                                  d[name][256 * t + 128 * i:256 * t + 128 * i + 128, :])
            tiles.append(wt)
        w8[name] = tiles
        return tiles

    wq1 = load_w8("Wq1")
    # rest of x8 (keys 512:2048)
    for t in range(XT):
        for i in range(2):
            nc.sync.dma_start(x8[t][:, i * S + QL:(i + 1) * S],
                              d["x8"][256 * t + 128 * i:256 * t + 128 * i + 128, QL:])

    def dr3(ap):
        """[128, 2*n] flat tile -> [128, 2, n] DoubleRow view."""
        return ap.rearrange("p (i c) -> p i c", i=2)

    # ---------------- building blocks ----------------
    def emit_q(wq, x_t, bq_key):
        qT = []
        for pair in range(PAIRS):
            qt = sbt([128, QL], BF16, "qT")
            acc = pst([128, 512], "acc")
            for t in range(XT):
                nc.tensor.matmul(acc, dr3(wq[t])[:, :, pair * 128:(pair + 1) * 128],
                                 dr3(x_t[t])[:, :, 0:QL],
                                 start=(t == 0), stop=(t == XT - 1), perf_mode=DR)
            nc.vector.tensor_scalar(qt, acc, 1.0 / 128.0, pap(pair, bq_key),
                                    op0=ALU.mult, op1=ALU.add)
            qT.append(qt)
        return qT

    def emit_k_pair(wk, x_t, bk_key, pair):
        kt_t = sbt([128, S], BF16, "kT")
        for c in range(S // 512):
            acc = pst([128, 512], "acc")
            for t in range(XT):
                nc.tensor.matmul(acc, dr3(wk[t])[:, :, pair * 128:(pair + 1) * 128],
                                 dr3(x_t[t])[:, :, c * 512:(c + 1) * 512],
                                 start=(t == 0), stop=(t == XT - 1), perf_mode=DR)
            nc.vector.tensor_scalar(kt_t[:, c * 512:(c + 1) * 512], acc,
                                    1.0 / 128.0, pap(pair, bk_key),
                                    op0=ALU.mult, op1=ALU.add)
        return kt_t

    def emit_v_all(wv, x_t):
        """8 ktpair v8 tiles [128, 2, 16h x 65] fp8 (= 32*v, ones col)."""
        v_ = []
        for m in range(KP):
            vt = sbt([128, 2 * H * (HD + 1)], FP8, "v")
            vv = vt.rearrange("p (i h c) -> p i h c", i=2, h=H)
            nc.vector.memset(vv[:, :, :, HD:HD + 1], 1.0)
            for half in range(2):
                for kt_ in (2 * m, 2 * m + 1):
                    acc = pst([128, 512], "acc")
                    for t in range(XT):
                        nc.tensor.matmul(
                            acc,
                            dr3(x_t[t])[:, :, kt_ * 128:(kt_ + 1) * 128],
                            dr3(wv[t])[:, :, half * 512:(half + 1) * 512],
                            start=(t == 0), stop=(t == XT - 1), perf_mode=DR)
                    av = acc.rearrange("p (h c) -> p h c", h=8)
                    nc.vector.tensor_scalar(
                        vv[:, kt_ % 2, half * 8:(half + 1) * 8, 0:HD], av,
                        0.25, None, op0=ALU.mult)
            v_.append(vt)
        return v_

    def make_k(wk, x_t, bk_key, n_upfront):
        kT_ = [emit_k_pair(wk, x_t, bk_key, p) for p in range(n_upfront)]

        def k_filler(pair_done):
            nxt = len(kT_)
            if nxt < PAIRS and nxt <= pair_done + 2:
                kT_.append(emit_k_pair(wk, x_t, bk_key, nxt))
        return kT_, k_filler

    def emit_attn(kT_t, v_t, qT_t, masked, o8_tiles, k_filler=None):
        """Fills o8_tiles (4 x [128, 2, 512] fp8 = 32*o, pair-major DR)."""
        for pair in range(PAIRS):
            pvs = [pst([HD + 1, QL], "pv") for _ in range(2)]
            prev = None  # pending ktpair PV
            for m in range(KP):
                p8 = sbt([128, 2048], FP8, "p")
                for kt_ in (2 * m, 2 * m + 1):
                    psc = pst([128, 2 * QL], "sc")
                    for half in range(2):
                        nc.tensor.matmul(
                            psc[:, half * QL:(half + 1) * QL],
                            kT_t[pair][half * HD:(half + 1) * HD,
                                       kt_ * 128:(kt_ + 1) * 128],
                            qT_t[pair][half * HD:(half + 1) * HD, :],
                            start=True, stop=True)
                    bias = bt_t[:, kt_:kt_ + 1] if masked else mln4_t[:, 0:1]
                    nc.scalar.activation(
                        p8[:, (kt_ % 2) * 1024:(kt_ % 2 + 1) * 1024],
                        psc, AF.Exp, bias=bias, scale=0.125)
                    if masked and kt_ < 4:
                        mt = sbt([128, 1024], FP8, "m")
                        nc.sync.dma_start(
                            mt, d["msk"][kt_ * 128:(kt_ + 1) * 128, :])
                        nc.vector.tensor_mul(
                            p8[:, (kt_ % 2) * 1024:(kt_ % 2 + 1) * 1024],
                            p8[:, (kt_ % 2) * 1024:(kt_ % 2 + 1) * 1024], mt)
                if prev is not None:
                    pp8, pm = prev
                    for half in range(2):
                        h = pair * 2 + half
                        nc.tensor.matmul(
                            pvs[half],
                            v_t[pm].rearrange(
                                "p (i h c) -> p i h c", i=2, h=H)[:, :, h:h + 1, :],
                            dr3(pp8)[:, :, half * 512:(half + 1) * 512],
                            start=(pm == 0), stop=(pm == KP - 1),
                            perf_mode=DR, skip_group_check=True)
                prev = (p8, m)
            pp8, pm = prev
            for half in range(2):
                h = pair * 2 + half
                nc.tensor.matmul(
                    pvs[half],
                    v_t[pm].rearrange(
                        "p (i h c) -> p i h c", i=2, h=H)[:, :, h:h + 1, :],
                    dr3(pp8)[:, :, half * 512:(half + 1) * 512],
                    start=(pm == 0), stop=(pm == KP - 1),
                    perf_mode=DR, skip_group_check=True)
            if k_filler is not None:
                k_filler(pair)
            ot = o8_tiles[pair // 2]
            for half in range(2):
                recip = sbt([1, QL], F32, "sm")
                nc.vector.reciprocal(recip, pvs[half][HD:HD + 1, :])
                rb = sbt([HD, QL], F32, "rb")
                nc.gpsimd.partition_broadcast(rb, recip)
                nc.vector.tensor_mul(
                    ot[half * HD:(half + 1) * HD,
                       (pair % 2) * 512:(pair % 2 + 1) * 512],
                    pvs[half][0:HD, :], rb)

    def emit_out_proj8(wo, o8_tiles, resid_t):
        """pre[dt] (f32) = Wo8.T @ o8 / 1024 + resid (bias folded in resid)"""
        pre = []
        for dt_ in range(DT):
            acc = pst([128, 512], "acc")
            for m in range(XT):
                nc.tensor.matmul(acc,
                                 dr3(wo[m])[:, :, dt_ * 128:(dt_ + 1) * 128],
                                 dr3(o8_tiles[m]),
                                 start=(m == 0), stop=(m == XT - 1),
                                 perf_mode=DR)
            t = sbt([128, QL], F32R, "res")
            nc.vector.scalar_tensor_tensor(t, acc, 1.0 / 1024.0, resid_t[dt_],
                                           op0=ALU.mult, op1=ALU.add)
            pre.append(t)
        return pre

    def emit_ffn(wf, s16, bias_key, resid_t):
        pre = []
        for dt_ in range(DT):
            acc = pst([128, 512], "acc")
            for pr in range(DT):
                nc.tensor.matmul(acc, wf[pr][:, dt_ * 128:(dt_ + 1) * 128],
                                 s16[pr], start=(pr == 0), stop=(pr == DT - 1))
            t = sbt([128, QL], F32R, "res")
            nc.vector.scalar_tensor_tensor(t, acc, pap(dt_, bias_key),
                                           resid_t[dt_],
                                           op0=ALU.add, op1=ALU.add)
            pre.append(t)
        return pre

    def emit_ln(pre_t, g_key, b_key, out8=None, out16=None):
        """LN over d (partitions+tiles).  out8: (tiles, gx4, bx4) for fp8 DR
        output; out16: bf16 output list."""
        xq_ = []
        for dt_ in range(DT):
            t2_ = sbt([128, QL], BF16, "xsq")
            nc.vector.tensor_mul(t2_, pre_t[dt_], pre_t[dt_])
            xq_.append(t2_)
        sx = pst([1, QL], "acc")
        for dt_ in range(DT):
            nc.tensor.matmul(sx, onesr_t, pre_t[dt_],
                             start=(dt_ == 0), stop=(dt_ == DT - 1),
                             skip_group_check=True)
        sxx = pst([1, QL], "acc")
        for dt_ in range(DT):
            nc.tensor.matmul(sxx, ones_t, xq_[dt_], start=(dt_ == 0),
                             stop=(dt_ == DT - 1), skip_group_check=True)
        mean = sbt([1, QL], F32, "sm")
        nc.vector.tensor_scalar(mean, sx, 1.0 / D, None, op0=ALU.mult)
        meanb = sbt([128, QL], F32, "smb")
        nc.gpsimd.partition_broadcast(meanb, mean)
        msq = sbt([1, QL], F32, "sm")
        nc.vector.tensor_mul(msq, mean, mean)
        var = sbt([1, QL], F32, "sm")
        nc.vector.scalar_tensor_tensor(var, sxx, 1.0 / D, msq,
                                       op0=ALU.mult, op1=ALU.subtract)
        sd = sbt([1, QL], F32, "sm")
        nc.scalar.activation(sd, var, AF.Sqrt, bias=eps_t)
        rstd = sbt([1, QL], F32, "sm")
        nc.vector.reciprocal(rstd, sd)
        rstdb = sbt([128, QL], F32, "smb")
        nc.gpsimd.partition_broadcast(rstdb, rstd)
        out32 = []
        for dt_ in range(DT):
            t1 = sbt([128, QL], F32, "t1")
            nc.vector.tensor_sub(t1, pre_t[dt_], meanb)
            t2_ = sbt([128, QL], F32, "t2")
            nc.vector.tensor_mul(t2_, t1, rstdb)
            o32 = sbt([128, QL], F32, "res")
            nc.vector.tensor_scalar(o32, t2_, pap(dt_, g_key), pap(dt_, b_key),
                                    op0=ALU.mult, op1=ALU.add)
            out32.append(o32)
            if out8 is not None:
                tiles, gk, bk = out8
                nc.vector.tensor_scalar(
                    tiles[dt_ // 2][:, (dt_ % 2) * 512:(dt_ % 2 + 1) * 512],
                    t2_, pap(dt_, gk), pap(dt_, bk), op0=ALU.mult, op1=ALU.add)
            if out16 is not None:
                o16 = sbt([128, QL], BF16, "sb16")
                nc.vector.tensor_scalar(o16, t2_, pap(dt_, g_key),
                                        pap(dt_, b_key), op0=ALU.mult,
                                        op1=ALU.add)
                out16.append(o16)
        return out32

    # ---------------- the decoder cell ----------------
    q1 = emit_q(wq1, x8, "bq1")

    wv1 = load_w8("Wv1")
    wk1 = load_w8("Wk1")
    x0r = []
    for dt_ in range(DT):
        t = sbt([128, QL], F32, "res")
        nc.sync.dma_start(t, d["x0r"][dt_ * 128:(dt_ + 1) * 128, :])
        x0r.append(t)

    v1 = emit_v_all(wv1, x8)
    k1, kf1 = make_k(wk1, x8, "bk1", 2)

    # h8 loads reuse xt slots (x8 dead after QKV1)
    h8 = []
    for t in range(XT):
        ht_ = sbt([128, 2 * S], FP8, "xt")
        for i in range(2):
            nc.sync.dma_start(ht_[:, i * S:(i + 1) * S],
                              d["h8"][256 * t + 128 * i:256 * t + 128 * i + 128, :])
        h8.append(ht_)

    o1 = [sbt([128, 1024], FP8, "o") for _ in range(XT)]
    emit_attn(k1, v1, q1, True, o1, k_filler=kf1)

    wv2 = load_w8("Wv2")
    v2 = emit_v_all(wv2, h8)

    wo1 = load_w8("Wo1")
    pre1 = emit_out_proj8(wo1, o1, x0r)
    s18 = [sbt([128, 1024], FP8, "s18") for _ in range(XT)]
    s1_32 = emit_ln(pre1, "g1", "b1o", out8=(s18, "g1x4", "b1x4"))

    wk2 = load_w8("Wk2")
    wq2 = load_w8("Wq2")
    k2, kf2 = make_k(wk2, h8, "bk2", 2)
    q2 = emit_q(wq2, s18, "bq2")

    o2 = [sbt([128, 1024], FP8, "o") for _ in range(XT)]
    emit_attn(k2, v2, q2, False, o2, k_filler=kf2)

    wo2 = load_w8("Wo2")
    pre2 = emit_out_proj8(wo2, o2, s1_32)
    s2_16 = []
    s2_32 = emit_ln(pre2, "g2", "b2", out16=s2_16)

    wf = []
    for dt_ in range(DT):
        t = sbt([128, D], BF16, "w")
        nc.sync.dma_start(t, d["Wf"][dt_ * 128:(dt_ + 1) * 128, :])
        wf.append(t)
    pre3 = emit_ffn(wf, s2_16, "bf", s2_32)
    s3_32 = emit_ln(pre3, "g3", "b3")

    for dt_ in range(DT):
        nc.sync.dma_start(d["out"][dt_ * 128:(dt_ + 1) * 128, :], s3_32[dt_])


_CACHE = {}


def build_program():
    if "nc" in _CACHE:
        return _CACHE["nc"]
    nc = bacc.Bacc("TRN2", target_bir_lowering=False, debug=False,
                   num_devices=NC)
    d = {}
    d["x8"] = nc.dram_tensor("x8", [D, S], FP8, kind="ExternalInput")
    d["h8"] = nc.dram_tensor("h8", [D, S], FP8, kind="ExternalInput")
    d["x0r"] = nc.dram_tensor("x0r", [D, QL], F32, kind="ExternalInput")
    d["msk"] = nc.dram_tensor("msk", [512, 1024], FP8, kind="ExternalInput")
    d["btbl"] = nc.dram_tensor("btbl", [128, KT], F32, kind="ExternalInput")
    d["onesr"] = nc.dram_tensor("onesr", [128, 1], F32R, kind="ExternalInput")
    for w in W8_NAMES:
        d[w] = nc.dram_tensor(w, [D, D], FP8, kind="ExternalInput")
    d["Wf"] = nc.dram_tensor("Wf", [D, D], BF16, kind="ExternalInput")
    d["par"] = nc.dram_tensor("par", [D, NPAR], F32, kind="ExternalInput")
    d["out"] = nc.dram_tensor("out", [D, QL], F32, kind="ExternalOutput")

    from contextlib import ExitStack
    with tile.TileContext(nc) as tc:
        with ExitStack() as ctx:
            _build_body(nc, tc, {k: (v[:] if hasattr(v, "ap") else v)
                                 for k, v in d.items()}, ctx)
    nc.compile()
    _CACHE["nc"] = nc
    return nc


def make_in_maps(inputs):
    """Build the 8 per-core input dicts from the full problem inputs."""
    f8 = ml_dtypes.float8_e4m3
    bf = ml_dtypes.bfloat16
    S0 = np.asarray(inputs["S0"], np.float32)
    Hh = np.asarray(inputs["H"], np.float32)

    def f32(name):
        return np.asarray(inputs[name], np.float32)

    par = np.zeros((D, NPAR), np.float32)
    par[:, PC["bq1"]] = f32("bq1")
    par[:, PC["bk1"]] = f32("bk1")
    par[:, PC["g1"]] = f32("ln1_g")
    # bo2 (+ folded bv2) rides on the LN1 f32 output used as attn2 residual
    par[:, PC["b1o"]] = (f32("ln1_b") + f32("bo2") + f32("bv2") @ f32("Wo2"))
    par[:, PC["g1x4"]] = 4.0 * f32("ln1_g")
    par[:, PC["b1x4"]] = 4.0 * f32("ln1_b")
    par[:, PC["bq2"]] = f32("bq2")
    par[:, PC["bk2"]] = f32("bk2")
    par[:, PC["g2"]] = f32("ln2_g")
    par[:, PC["b2"]] = f32("ln2_b")
    par[:, PC["bf"]] = f32("bf")
    par[:, PC["g3"]] = f32("ln3_g")
    par[:, PC["b3"]] = f32("ln3_b")

    ws8 = {w: np.ascontiguousarray(f32(w) * 32.0).astype(f8) for w in W8_NAMES}
    wf16 = np.ascontiguousarray(f32("Wf")).astype(bf)
    # bo1 (+ folded bv1) rides on the x0 residual
    rbias = (f32("bo1") + f32("bv1") @ f32("Wo1"))

    in_maps = []
    for c in range(NC):
        b, j = c // 4, c % 4
        q0 = j * QL
        # key permutation: own diagonal block first, then visible, then masked
        perm = np.concatenate([
            np.arange(q0, q0 + QL),          # kt 0..3  (diag block)
            np.arange(0, q0),                # kt 4..4j+3 (fully visible)
            np.arange(q0 + QL, S),           # rest (fully masked)
        ]).astype(np.int64)
        x0t = np.ascontiguousarray(S0[b].T)          # [D, S]
        x8 = np.ascontiguousarray(x0t[:, perm] * 4.0).astype(f8)
        h8 = np.ascontiguousarray(Hh[b].T * 4.0).astype(f8)
        # triangular mask for the diagonal block (keys q0..q0+512 vs own rows)
        mask = (perm[:QL, None] <= (q0 + np.arange(QL))[None, :])
        msk = np.concatenate([mask, mask], axis=1).astype(f8)  # [512, 1024]
        btbl = np.zeros((128, KT), np.float32)
        btbl[:, :4 + 4 * j] = MLN4
        btbl[:, 4 + 4 * j:] = -100.0
        x0r = np.ascontiguousarray(x0t[:, q0:q0 + QL]) + rbias[:, None]
        m = {
            "x8": x8,
            "h8": h8,
            "x0r": x0r.astype(np.float32),
            "msk": msk,
            "btbl": btbl,
            "onesr": np.ones((128, 1), np.float32),
            "Wf": wf16,
            "par": par,
        }
        m.update(ws8)
        in_maps.append(m)
    return in_maps


def kernel(**inputs) -> np.ndarray:
    from concourse.bass_utils import run_bass_kernel_spmd
    nc = build_program()
    in_maps = make_in_maps(inputs)
    res = run_bass_kernel_spmd(nc, in_maps, list(range(NC)))
    _CACHE["last_results"] = res
    out = np.zeros((B, S, D), np.float32)
    for c in range(NC):
        b, j = c // 4, c % 4
        out[b, j * QL:(j + 1) * QL, :] = res.results[c]["out"].T
    return out
